# revision 6
# baseline (speedup 1.0000x reference)
"""Trainium2 Bass kernel for nn_DiffLoss (diff loss over per-(b,s) means).

reference:
    mc = mean(common, axis=(2,3,4))   # [B,S]
    ms = mean(specific, axis=(2,3,4)) # [B,S]
    out = mean(mc * ms)               # scalar

Shapes: common/specific [16, 12, 64, 64, 64] f32.

Strategy (pure data parallel, batch sharded 8 ways, 2 batches per core):
  Each core streams its 2x12x64x64x64 shard of both tensors (50.3 MB total)
  through SBUF in 4 MB tiles ([128, 8192] f32) via HWDGE DMAs and does a
  free-dim reduce_sum per tile on the vector engine.  One (b,s) pair is
  262144 contiguous elements = 32 partitions x 8192, so each [128, 8192]
  tile covers exactly 4 pairs and per-pair sums are recoverable from the
  per-partition column sums.  The kernel writes the [128, 12] column sums
  (6 tiles x 2 tensors) to DRAM; the host finishes the tiny O(1500)-element
  reduction in float64.  This keeps the device kernel purely DMA-bound at
  the HBM roofline (~140 us/core).
"""

import numpy as np

# Problem constants (hardcoded; kernel.py must be self-contained).
B, S, C, H, W = 16, 12, 64, 64, 64
N_CORES = 8
B_LOCAL = B // N_CORES          # 2 batches per core
PAIRS_PER_CORE = B_LOCAL * S    # 24 (b,s) pairs per core
PAIR_ELEMS = C * H * W          # 262144 elements per pair
P = 128                         # SBUF partitions
F = 8192                        # free elems per partition per tile (4 MB tile)
NT = (PAIRS_PER_CORE * PAIR_ELEMS) // (P * F)   # 6 tiles per tensor per core
PAIRS_PER_TILE = (P * F) // PAIR_ELEMS          # 4
PART_PER_PAIR = P // PAIRS_PER_TILE             # 32 partitions per pair

_CACHE = {}


def _build_bass(reps=1):
    import concourse.bass as bass
    import concourse.mybir as mybir
    import concourse.tile as tile
    from concourse import bacc

    nc = bacc.Bacc("TRN2", target_bir_lowering=False, debug=False,
                   num_devices=N_CORES)
    xc = nc.dram_tensor("common", (NT, P, F), mybir.dt.float32,
                        kind="ExternalInput").ap()
    xs = nc.dram_tensor("specific", (NT, P, F), mybir.dt.float32,
                        kind="ExternalInput").ap()
    out = nc.dram_tensor("colsums", (P, 2 * NT), mybir.dt.float32,
                         kind="ExternalOutput").ap()

    with tile.TileContext(nc) as tc:
        with tc.tile_pool(name="io", bufs=4) as pool, \
             tc.tile_pool(name="stat", bufs=1) as spool:
            acc = spool.tile([P, 2 * NT], mybir.dt.float32)
            for _ in range(reps):
                for t in range(NT):
                    for j, src in enumerate((xc, xs)):
                        tl = pool.tile([P, F], mybir.dt.float32, tag="stream")
                        nc.gpsimd.dma_start(tl[:], src[t, :, :])
                        nc.vector.reduce_sum(
                            acc[:, j * NT + t: j * NT + t + 1], tl[:],
                            axis=mybir.AxisListType.X)
            # Funnel the 12 column writes through one same-engine copy so the
            # output DMA carries a single sync wait (HW limits waits per inst).
            accc = spool.tile([P, 2 * NT], mybir.dt.float32)
            nc.vector.tensor_copy(accc[:], acc[:])
            nc.sync.dma_start(out[:], accc[:])
    nc.compile()
    return nc


def _get_bass():
    if "nc" not in _CACHE:
        _CACHE["nc"] = _build_bass()
    return _CACHE["nc"]


def _make_in_maps(common, specific):
    in_maps = []
    for c in range(N_CORES):
        in_maps.append({
            "common": np.ascontiguousarray(
                common[c * B_LOCAL:(c + 1) * B_LOCAL]).reshape(NT, P, F),
            "specific": np.ascontiguousarray(
                specific[c * B_LOCAL:(c + 1) * B_LOCAL]).reshape(NT, P, F),
        })
    return in_maps


def _finish_host(results):
    """Combine per-core [128, 12] column sums into the final scalar."""
    total = 0.0
    for c in range(N_CORES):
        cs = np.asarray(results[c]["colsums"], dtype=np.float64)  # [128, 12]
        comm = cs[:, :NT]      # [128, 6]  col t = tile t of `common`
        spec = cs[:, NT:]      # [128, 6]  col t = tile t of `specific`
        # pair(t, p) = 4*t + p//32  ->  reshape partitions to [4, 32]
        sc = comm.reshape(PAIRS_PER_TILE, PART_PER_PAIR, NT).sum(axis=1)
        ss = spec.reshape(PAIRS_PER_TILE, PART_PER_PAIR, NT).sum(axis=1)
        total += float((sc * ss).sum())
    n = float(PAIR_ELEMS)
    result = total / (n * n) / float(B * S)
    return np.asarray(result, dtype=np.float32)


def kernel(common, specific, _trace=False):
    from concourse import bass_utils

    common = np.asarray(common, dtype=np.float32)
    specific = np.asarray(specific, dtype=np.float32)
    assert common.shape == (B, S, C, H, W)
    assert specific.shape == (B, S, C, H, W)

    nc = _get_bass()
    in_maps = _make_in_maps(common, specific)
    res = bass_utils.run_bass_kernel_spmd(
        nc, in_maps, core_ids=list(range(N_CORES)), trace=_trace)
    _CACHE["last_results"] = res
    return _finish_host(res.results)


if __name__ == "__main__":
    rng = np.random.default_rng(0)
    common = rng.standard_normal((B, S, C, H, W), dtype=np.float32)
    specific = rng.standard_normal((B, S, C, H, W), dtype=np.float32)
    got = kernel(common, specific)
    mc = common.reshape(B, S, -1).mean(axis=2, dtype=np.float64)
    ms = specific.reshape(B, S, -1).mean(axis=2, dtype=np.float64)
    want = (mc * ms).mean()
    print("got", got, "want", want, "rel", abs(got - want) / abs(want))


# revision 13
# speedup vs baseline: 1.1929x; 1.1929x over previous
"""Trainium2 Bass kernel for nn_DiffLoss (diff loss over per-(b,s) means).

reference:
    mc = mean(common, axis=(2,3,4))   # [B,S]
    ms = mean(specific, axis=(2,3,4)) # [B,S]
    out = mean(mc * ms)               # scalar

Shapes: common/specific [16, 12, 64, 64, 64] f32.

Strategy (pure data parallel, batch sharded 8 ways, 2 batches per core):
  Each core streams its 2x12x64x64x64 shard of both tensors (50.3 MB total)
  through SBUF in 4 MB tiles ([128, 8192] f32) via HWDGE DMAs (alternating
  the qSP/qAct dynamic rings) and does a free-dim reduce_sum per tile on the
  vector engine.  One (b,s) pair is 262144 contiguous elements =
  32 partitions x 8192, so each [128, 8192] tile covers exactly 4 pairs and
  per-pair sums are recoverable from the per-partition column sums.  The
  kernel writes the [128, 12] column sums (6 tiles x 2 tensors) to DRAM; the
  host finishes the tiny O(1500)-element reduction in float64.  This keeps
  the device kernel purely DMA-bound at the per-core HBM roofline:
  measured ~141-146 us/core vs 50.33 MB / 358 GB/s = 140.6 us (97-100%).
"""

import numpy as np

# Problem constants (hardcoded; kernel.py must be self-contained).
B, S, C, H, W = 16, 12, 64, 64, 64
N_CORES = 8
B_LOCAL = B // N_CORES          # 2 batches per core
PAIRS_PER_CORE = B_LOCAL * S    # 24 (b,s) pairs per core
PAIR_ELEMS = C * H * W          # 262144 elements per pair
P = 128                         # SBUF partitions
F = 8192                        # free elems per partition per tile (4 MB tile)
NT = (PAIRS_PER_CORE * PAIR_ELEMS) // (P * F)   # 6 tiles per tensor per core
PAIRS_PER_TILE = (P * F) // PAIR_ELEMS          # 4
PART_PER_PAIR = P // PAIRS_PER_TILE             # 32 partitions per pair

_CACHE = {}


def _build_bass(reps=1, f=F, bufs=4, engines=("sync", "scalar"), loop_reps=0):
    """Build the per-core Bass program.

    reps: static unrolled repetitions of the full streaming pass (bench only;
        each rep writes its own output columns so DCE can't drop it).
    loop_reps: if >0, additionally wrap the body in a dynamic For_i loop that
        executes it loop_reps times (bench only; amortizes dispatch overhead
        for wall-clock timing).
    """
    import concourse.bass as bass
    import concourse.mybir as mybir
    import concourse.tile as tile
    from concourse import bacc

    nt = (PAIRS_PER_CORE * PAIR_ELEMS) // (P * f)  # tiles per tensor
    nc = bacc.Bacc("TRN2", target_bir_lowering=False, debug=False,
                   num_devices=N_CORES)
    xc = nc.dram_tensor("common", (nt, P, f), mybir.dt.float32,
                        kind="ExternalInput").ap()
    xs = nc.dram_tensor("specific", (nt, P, f), mybir.dt.float32,
                        kind="ExternalInput").ap()
    out = nc.dram_tensor("colsums", (P, 2 * nt * reps), mybir.dt.float32,
                         kind="ExternalOutput").ap()

    with tile.TileContext(nc) as tc:
        with tc.tile_pool(name="io", bufs=bufs) as pool, \
             tc.tile_pool(name="stat", bufs=1) as spool:
            acc = spool.tile([P, 2 * nt * reps], mybir.dt.float32)

            def body():
                k = 0
                for r in range(reps):
                    for t in range(nt):
                        for j, src in enumerate((xc, xs)):
                            tl = pool.tile([P, f], mybir.dt.float32,
                                           tag="stream", name="tl")
                            eng = getattr(nc, engines[k % len(engines)])
                            k += 1
                            eng.dma_start(tl[:], src[t, :, :])
                            col = 2 * nt * r + j * nt + t
                            nc.vector.reduce_sum(
                                acc[:, col: col + 1], tl[:],
                                axis=mybir.AxisListType.X)

            if loop_reps > 0:
                with tc.For_i(0, loop_reps, 1):
                    body()
            else:
                body()
            # Funnel the column writes through one same-engine copy so the
            # output DMA carries a single sync wait (HW limits waits per inst).
            accc = spool.tile([P, 2 * nt * reps], mybir.dt.float32)
            nc.vector.tensor_copy(accc[:], acc[:])
            nc.sync.dma_start(out[:], accc[:])
    nc.compile()
    return nc


def _get_bass():
    if "nc" not in _CACHE:
        _CACHE["nc"] = _build_bass()
    return _CACHE["nc"]


def _make_in_maps(common, specific, f=F):
    nt = (PAIRS_PER_CORE * PAIR_ELEMS) // (P * f)
    in_maps = []
    for c in range(N_CORES):
        in_maps.append({
            "common": np.ascontiguousarray(
                common[c * B_LOCAL:(c + 1) * B_LOCAL]).reshape(nt, P, f),
            "specific": np.ascontiguousarray(
                specific[c * B_LOCAL:(c + 1) * B_LOCAL]).reshape(nt, P, f),
        })
    return in_maps


def _finish_host(results, f=F):
    """Combine per-core [128, 2*nt] column sums into the final scalar."""
    nt = (PAIRS_PER_CORE * PAIR_ELEMS) // (P * f)
    pairs_per_tile = (P * f) // PAIR_ELEMS
    part_per_pair = P // pairs_per_tile
    total = 0.0
    for c in range(N_CORES):
        cs = np.asarray(results[c]["colsums"], dtype=np.float64)  # [128, 2*nt*reps]
        comm = cs[:, :nt]           # col t = tile t of `common`
        spec = cs[:, nt:2 * nt]     # col t = tile t of `specific`
        # pair(t, p) = pairs_per_tile*t + p//part_per_pair
        sc = comm.reshape(pairs_per_tile, part_per_pair, nt).sum(axis=1)
        ss = spec.reshape(pairs_per_tile, part_per_pair, nt).sum(axis=1)
        total += float((sc * ss).sum())
    n = float(PAIR_ELEMS)
    result = total / (n * n) / float(B * S)
    return np.asarray(result, dtype=np.float32)


def kernel(common, specific, _trace=False):
    from concourse import bass_utils

    common = np.asarray(common, dtype=np.float32)
    specific = np.asarray(specific, dtype=np.float32)
    assert common.shape == (B, S, C, H, W)
    assert specific.shape == (B, S, C, H, W)

    nc = _get_bass()
    in_maps = _make_in_maps(common, specific)
    res = bass_utils.run_bass_kernel_spmd(
        nc, in_maps, core_ids=list(range(N_CORES)), trace=_trace)
    _CACHE["last_results"] = res
    return _finish_host(res.results)


if __name__ == "__main__":
    rng = np.random.default_rng(0)
    common = rng.standard_normal((B, S, C, H, W), dtype=np.float32)
    specific = rng.standard_normal((B, S, C, H, W), dtype=np.float32)
    got = kernel(common, specific)
    mc = common.reshape(B, S, -1).mean(axis=2, dtype=np.float64)
    ms = specific.reshape(B, S, -1).mean(axis=2, dtype=np.float64)
    want = (mc * ms).mean()
    print("got", got, "want", want, "rel", abs(got - want) / abs(want))


# revision 14
# speedup vs baseline: 1.4346x; 1.2026x over previous
"""Trainium2 Bass kernel for nn_DiffLoss (diff loss over per-(b,s) means).

reference:
    mc = mean(common, axis=(2,3,4))   # [B,S]
    ms = mean(specific, axis=(2,3,4)) # [B,S]
    out = mean(mc * ms)               # scalar

Shapes: common/specific [16, 12, 64, 64, 64] f32.

Strategy (pure data parallel, batch sharded 8 ways, 2 batches per core):
  Each core streams its 2x12x64x64x64 shard of both tensors (50.3 MB total)
  through SBUF in 4 MB tiles ([128, 8192] f32) via HWDGE DMAs (alternating
  the qSP/qAct dynamic rings) and does a free-dim reduce_sum per tile on the
  vector engine.  One (b,s) pair is 262144 contiguous elements =
  32 partitions x 8192, so each [128, 8192] tile covers exactly 4 pairs and
  per-pair sums are recoverable from the per-partition column sums.  The
  kernel writes the [128, 12] column sums (6 tiles x 2 tensors) to DRAM; the
  host finishes the tiny O(1500)-element reduction in float64.  This keeps
  the device kernel purely DMA-bound at the per-core HBM roofline:
  measured ~141-146 us/core vs 50.33 MB / 358 GB/s = 140.6 us (97-100%).
"""

import numpy as np

# Problem constants (hardcoded; kernel.py must be self-contained).
B, S, C, H, W = 16, 12, 64, 64, 64
N_CORES = 8
B_LOCAL = B // N_CORES          # 2 batches per core
PAIRS_PER_CORE = B_LOCAL * S    # 24 (b,s) pairs per core
PAIR_ELEMS = C * H * W          # 262144 elements per pair
P = 128                         # SBUF partitions
F = 8192                        # free elems per partition per tile (4 MB tile)
NT = (PAIRS_PER_CORE * PAIR_ELEMS) // (P * F)   # 6 tiles per tensor per core
PAIRS_PER_TILE = (P * F) // PAIR_ELEMS          # 4
PART_PER_PAIR = P // PAIRS_PER_TILE             # 32 partitions per pair

_CACHE = {}


def _build_bass(reps=1, f=F, bufs=4, engines=("sync", "scalar"), loop_reps=0):
    """Build the per-core Bass program.

    reps: static unrolled repetitions of the full streaming pass (bench only;
        each rep writes its own output columns so DCE can't drop it).
    loop_reps: if >0, additionally wrap the body in a dynamic For_i loop that
        executes it loop_reps times (bench only; amortizes dispatch overhead
        for wall-clock timing).
    """
    import concourse.bass as bass
    import concourse.mybir as mybir
    import concourse.tile as tile
    from concourse import bacc

    nt = (PAIRS_PER_CORE * PAIR_ELEMS) // (P * f)  # tiles per tensor
    nc = bacc.Bacc("TRN2", target_bir_lowering=False, debug=False,
                   num_devices=N_CORES)
    xc = nc.dram_tensor("common", (nt, P, f), mybir.dt.float32,
                        kind="ExternalInput").ap()
    xs = nc.dram_tensor("specific", (nt, P, f), mybir.dt.float32,
                        kind="ExternalInput").ap()
    out = nc.dram_tensor("colsums", (P, 2 * nt * reps), mybir.dt.float32,
                         kind="ExternalOutput").ap()

    with tile.TileContext(nc) as tc:
        with tc.tile_pool(name="io", bufs=bufs) as pool, \
             tc.tile_pool(name="stat", bufs=1) as spool:
            acc = spool.tile([P, 2 * nt * reps], mybir.dt.float32)

            def body():
                k = 0
                for r in range(reps):
                    for t in range(nt):
                        for j, src in enumerate((xc, xs)):
                            tl = pool.tile([P, f], mybir.dt.float32,
                                           tag="stream", name="tl")
                            eng = getattr(nc, engines[k % len(engines)])
                            k += 1
                            eng.dma_start(tl[:], src[t, :, :])
                            col = 2 * nt * r + j * nt + t
                            nc.vector.reduce_sum(
                                acc[:, col: col + 1], tl[:],
                                axis=mybir.AxisListType.X)

            if loop_reps > 0:
                with tc.For_i(0, loop_reps, 1):
                    body()
            else:
                body()
            # Funnel the column writes through one same-engine copy so the
            # output DMA carries a single sync wait (HW limits waits per inst).
            accc = spool.tile([P, 2 * nt * reps], mybir.dt.float32)
            nc.vector.tensor_copy(accc[:], acc[:])
            nc.sync.dma_start(out[:], accc[:])
    nc.compile()
    return nc


def _get_bass():
    if "nc" not in _CACHE:
        _CACHE["nc"] = _build_bass()
    return _CACHE["nc"]


def _make_in_maps(common, specific, f=F):
    nt = (PAIRS_PER_CORE * PAIR_ELEMS) // (P * f)
    in_maps = []
    for c in range(N_CORES):
        in_maps.append({
            "common": np.ascontiguousarray(
                common[c * B_LOCAL:(c + 1) * B_LOCAL]).reshape(nt, P, f),
            "specific": np.ascontiguousarray(
                specific[c * B_LOCAL:(c + 1) * B_LOCAL]).reshape(nt, P, f),
        })
    return in_maps


def _finish_host(results, f=F):
    """Combine per-core [128, 2*nt] column sums into the final scalar."""
    nt = (PAIRS_PER_CORE * PAIR_ELEMS) // (P * f)
    pairs_per_tile = (P * f) // PAIR_ELEMS
    part_per_pair = P // pairs_per_tile
    total = 0.0
    for c in range(N_CORES):
        cs = np.asarray(results[c]["colsums"], dtype=np.float64)  # [128, 2*nt*reps]
        comm = cs[:, :nt]           # col t = tile t of `common`
        spec = cs[:, nt:2 * nt]     # col t = tile t of `specific`
        # pair(t, p) = pairs_per_tile*t + p//part_per_pair
        sc = comm.reshape(pairs_per_tile, part_per_pair, nt).sum(axis=1)
        ss = spec.reshape(pairs_per_tile, part_per_pair, nt).sum(axis=1)
        total += float((sc * ss).sum())
    n = float(PAIR_ELEMS)
    result = total / (n * n) / float(B * S)
    return np.asarray(result, dtype=np.float32)


def kernel(common, specific, _trace=False):
    from concourse import bass_utils

    common = np.asarray(common, dtype=np.float32)
    specific = np.asarray(specific, dtype=np.float32)
    assert common.shape == (B, S, C, H, W)
    assert specific.shape == (B, S, C, H, W)

    nc = _get_bass()
    in_maps = _make_in_maps(common, specific)
    last_exc = None
    for attempt in range(3):
        try:
            res = bass_utils.run_bass_kernel_spmd(
                nc, in_maps, core_ids=list(range(N_CORES)), trace=_trace)
            break
        except Exception as e:  # transient axon/NRT wedges recover on retry
            last_exc = e
            if attempt == 2:
                raise
            import time
            time.sleep(5)
    _CACHE["last_results"] = res
    return _finish_host(res.results)


if __name__ == "__main__":
    rng = np.random.default_rng(0)
    common = rng.standard_normal((B, S, C, H, W), dtype=np.float32)
    specific = rng.standard_normal((B, S, C, H, W), dtype=np.float32)
    got = kernel(common, specific)
    mc = common.reshape(B, S, -1).mean(axis=2, dtype=np.float64)
    ms = specific.reshape(B, S, -1).mean(axis=2, dtype=np.float64)
    want = (mc * ms).mean()
    print("got", got, "want", want, "rel", abs(got - want) / abs(want))
